# revision 22
# baseline (speedup 1.0000x reference)
"""nn_CrossAttention — Trainium2 Bass kernel (8 NeuronCores, SPMD), v2.

Sharding: core c handles batch b=c//2 and head-group g=c%2 (4 of 8 heads).
Host-side unshard sums the two head-group partials per batch, transposes,
and adds the output bias.

v2 engine plan (vs the f32r v1):
- Q/K/V/Y projections in bf16 (full PE rate, tiny error).
- Scores (Q.K^T) and attention.V in fp8e4 with DoubleRow perf mode
  (0.5 cycles/row).  Scores zero-pad the second contraction slot (dh=64);
  AV uses the two slots for adjacent key blocks (real 256-wide contraction).
- exp() split across engines: most tiles on Act (table exp), some via a
  Schraudolph bit-trick on DVE (f32 affine -> int16 == bf16 bits) with the
  bf16->fp8 quantize pass on GPSIMD/Pool.
- Softmax denominators from a ones column appended to V (row 64 of the AV
  PSUM accumulator); no max-subtraction (scores are O(1) here).
- y streamed directly PSUM -> DRAM (f32), x/weights DMA'd in bf16.
"""

from contextlib import ExitStack

import numpy as np
import ml_dtypes

import concourse.bass as bass
import concourse.mybir as mybir
import concourse.tile as tile
from concourse import bacc
from concourse.bass_utils import run_bass_kernel_spmd

F32 = mybir.dt.float32
BF16 = mybir.dt.bfloat16
FP8 = mybir.dt.float8e4
U8 = mybir.dt.uint8
F16 = mybir.dt.float16
DR = mybir.MatmulPerfMode.DoubleRow

B, H = 4, 8
N = 4096          # queries per batch
M = 1024          # keys
QD = 1024         # query dim
CD = 768          # context dim
DH = 64           # head dim
HL = 4            # heads per core
IL = HL * DH      # local inner = 256
SCALE = DH ** -0.5

QCH = 512         # query chunk (moving dim)
NCH = N // QCH
NKT = M // 128
NQT = QD // 128
NCT = CD // 128
NYT = QD // 128
VW = 80        # padded V width for dual-fp8 ldweights (multiple of 16)



def _build():
    nc = bacc.Bacc("TRN2", target_bir_lowering=False, debug=False)

    xT = nc.declare_dram_parameter("xT", [QD, N], BF16, isOutput=False)
    ctxT = nc.declare_dram_parameter("ctxT", [CD, M], BF16, isOutput=False)
    wq = nc.declare_dram_parameter("wq", [QD, IL], BF16, isOutput=False)
    wk = nc.declare_dram_parameter("wk", [CD, IL], BF16, isOutput=False)
    wv = nc.declare_dram_parameter("wv", [CD, IL], BF16, isOutput=False)
    wo = nc.declare_dram_parameter("wo", [IL, QD], BF16, isOutput=False)
    yT = nc.declare_dram_parameter("yT", [QD, N], BF16, isOutput=True)

    xT_r = xT.rearrange("(kt p) (c q) -> p kt c q", p=128, q=QCH)
    ctx_r = ctxT.rearrange("(ct p) m -> p ct m", p=128)
    wq_r = wq.rearrange("(kt p) i -> p kt i", p=128)
    wk_r = wk.rearrange("(ct p) i -> p ct i", p=128)
    wv_r = wv.rearrange("(ct p) i -> p ct i", p=128)
    wo_r = wo.rearrange("(it p) d -> p it d", p=128)
    yT_r = yT.rearrange("(yt p) (c q) -> p yt c q", p=128, q=QCH)

    with tile.TileContext(nc) as tc, ExitStack() as stack:
        sing = stack.enter_context(tc.tile_pool(name="sing", bufs=1))

        # ---- persistent tiles ----
        wq_sb = sing.tile([128, NQT, IL], BF16)
        nc.sync.dma_start(out=wq_sb, in_=wq_r)
        wo_sb = sing.tile([128, 2, QD], BF16)
        nc.sync.dma_start(out=wo_sb, in_=wo_r)
        # K^T in fp8: [dh-of-pair, mi(head pair), kt, slot, kcol]; the dual-fp8
        # Ldweights needs the two slots contiguous, so slot sits next to kcol.
        # Slot 1 is zeroed (scores contract only dh=64).
        kt8 = sing.tile([128, 2, NKT, 2, 128], FP8)
        nc.vector.memset(kt8[:, :, :, 1, :], 0.0)
        # V in fp16 with a ones column (softmax denom = AV psum row 64):
        # [key%128, kt, head, dh+1]
        vaug = sing.tile([128, NKT, HL, DH + 1], F16)
        ones32 = sing.tile([128, NKT, HL, 1], F32)
        nc.vector.memset(ones32, 1.0)
        nc.vector.tensor_copy(vaug[:, :, :, DH:DH + 1], ones32)
        # Q^T in fp8, manual double buffer: [dh-of-pair, buf, mi, slot, q]
        qt2 = sing.tile([128, 2, 2, 2, QCH], FP8)
        nc.vector.memset(qt2[:, :, :, 1, :], 0.0)

        # ---- stage A: compute K^T and V_aug ----
        with tc.tile_pool(name="stagea", bufs=1) as stagea, \
             tc.tile_pool(name="psa_a", bufs=2, space="PSUM") as psa_a:
            wk_sb = stagea.tile([128, NCT, IL], BF16)
            nc.sync.dma_start(out=wk_sb, in_=wk_r)
            wv_sb = stagea.tile([128, NCT, IL], BF16)
            nc.sync.dma_start(out=wv_sb, in_=wv_r)
            ctx_sb = stagea.tile([128, NCT, M], BF16)
            nc.sync.dma_start(out=ctx_sb, in_=ctx_r)

            for mi in range(2):
                for nch2 in range(M // QCH):
                    pk = psa_a.tile([128, QCH], F32, tag="ps1")
                    for ct in range(NCT):
                        nc.tensor.matmul(
                            pk, wk_sb[:, ct, mi * 128:(mi + 1) * 128],
                            ctx_sb[:, ct, nch2 * QCH:(nch2 + 1) * QCH],
                            start=(ct == 0), stop=(ct == NCT - 1))
                    nc.vector.tensor_copy(
                        kt8[:, mi, nch2 * 4:(nch2 + 1) * 4, 0, :],
                        pk.rearrange("p (k c) -> p k c", k=4))
            for kt in range(NKT):
                pv = psa_a.tile([128, IL], F32, tag="ps1")
                for ct in range(NCT):
                    nc.tensor.matmul(
                        pv, ctx_sb[:, ct, kt * 128:(kt + 1) * 128],
                        wv_sb[:, ct, :],
                        start=(ct == 0), stop=(ct == NCT - 1))
                nc.vector.tensor_copy(
                    vaug[:, kt, :, 0:DH],
                    pv.rearrange("p (h d) -> p h d", h=HL))

        # ---- stage B pools ----
        xpool = stack.enter_context(tc.tile_pool(name="xpool", bufs=2))
        ptp = stack.enter_context(tc.tile_pool(name="ptp", bufs=2))
        o2p = stack.enter_context(tc.tile_pool(name="o2p", bufs=3))
        ycp = stack.enter_context(tc.tile_pool(name="ycp", bufs=3))
        smallp = stack.enter_context(tc.tile_pool(name="smallp", bufs=2))
        ps2 = stack.enter_context(tc.tile_pool(name="ps2", bufs=3, space="PSUM"))
        pso = stack.enter_context(tc.tile_pool(name="pso", bufs=2, space="PSUM"))

        for c in range(NCH):
            buf = c % 2
            xc = xpool.tile([128, NQT, QCH], BF16)
            nc.sync.dma_start(out=xc, in_=xT_r[:, :, c, :])

            # Q projection: both head pairs into one 2-bank psum tile
            pq = ps2.tile([128, 2, QCH], F32, tag="s")
            for mi in range(2):
                for kt in range(NQT):
                    nc.tensor.matmul(
                        pq[:, mi, :], wq_sb[:, kt, mi * 128:(mi + 1) * 128],
                        xc[:, kt, :],
                        start=(kt == 0), stop=(kt == NQT - 1))
            nc.vector.tensor_copy(qt2[:, buf, :, 0, :], pq)

            # software-pipelined heads: emit scores+exp for head h, then the
            # AV+normalize for head h-1 (whose exps overlap head h's scores)
            o2a = o2p.tile([128, QCH], BF16, tag="o2a")
            o2b = o2p.tile([128, QCH], BF16, tag="o2b")
            o2t = [o2a, o2b]
            pts = {}

            def scores_head(h):
                mi, hoff = h // 2, (h % 2) * 64
                pt = ptp.tile([128, NKT, QCH], F16)
                pts[h] = pt
                for j in range(4):  # kt pairs
                    sc = ps2.tile([128, 2, QCH], F32, tag="s")
                    for i in range(2):
                        kt = 2 * j + i
                        nc.tensor.matmul(
                            sc[:, i, :],
                            kt8[hoff:hoff + 64, mi, kt, :, :],
                            qt2[hoff:hoff + 64, buf, mi, :, :],
                            start=True, stop=True, perf_mode=DR)
                    nc.scalar.activation(
                        pt[:, 2 * j:2 * j + 2, :], sc,
                        mybir.ActivationFunctionType.Exp, scale=SCALE)

            def av_head(h):
                hoff = (h % 2) * 64
                pt = pts.pop(h)
                po = pso.tile([DH + 1, QCH], F32)
                for kt in range(NKT):
                    nc.tensor.matmul(
                        po, vaug[:, kt, h, :], pt[:, kt, :],
                        start=(kt == 0), stop=(kt == NKT - 1))
                ra = smallp.tile([1, QCH], F32, tag=f"ra{h % 2}")
                nc.vector.reciprocal(ra, po[DH:DH + 1, :])
                bca = smallp.tile([64, QCH], F32, tag=f"bc{h % 2}")
                nc.gpsimd.partition_broadcast(bca, ra)
                nc.vector.tensor_mul(o2t[h // 2][hoff:hoff + 64, :],
                                     po[0:DH, :], bca)

            scores_head(0)
            for h in range(1, HL):
                scores_head(h)
                av_head(h - 1)
            av_head(HL - 1)

            for yp in range(NYT // 2):
                py = ps2.tile([128, 2, QCH], F32, tag="s")
                for i in range(2):
                    yt = 2 * yp + i
                    nc.tensor.matmul(
                        py[:, i, :], wo_sb[:, 0, yt * 128:(yt + 1) * 128],
                        o2t[0], start=True, stop=False)
                    nc.tensor.matmul(
                        py[:, i, :], wo_sb[:, 1, yt * 128:(yt + 1) * 128],
                        o2t[1], start=False, stop=True)
                yc = ycp.tile([128, 2, QCH], BF16)
                nc.vector.tensor_copy(yc, py)
                nc.sync.dma_start(
                    out=yT_r[:, 2 * yp:2 * yp + 2, c, :], in_=yc)

    nc.compile()
    return nc


_NC_CACHE = {}


def _get_nc():
    if "nc" not in _NC_CACHE:
        _NC_CACHE["nc"] = _build()
    return _NC_CACHE["nc"]


def kernel(x, context, Wq, Wk, Wv, Wo, bo):
    bf = ml_dtypes.bfloat16
    x = np.asarray(x, np.float32)
    context = np.asarray(context, np.float32)
    Wq = np.asarray(Wq, np.float32)
    Wk = np.asarray(Wk, np.float32)
    Wv = np.asarray(Wv, np.float32)
    Wo = np.asarray(Wo, np.float32)
    bo = np.asarray(bo, np.float32)

    nc = _get_nc()
    in_maps = []
    for c in range(8):
        b, g = c // 2, c % 2
        sl = slice(g * IL, (g + 1) * IL)
        in_maps.append({
            "xT": np.ascontiguousarray(x[b].T).astype(bf),
            "ctxT": np.ascontiguousarray(context[b].T).astype(bf),
            "wq": np.ascontiguousarray(Wq[:, sl]).astype(bf),
            "wk": np.ascontiguousarray(Wk[:, sl]).astype(bf),
            "wv": np.ascontiguousarray(Wv[:, sl]).astype(bf),
            "wo": np.ascontiguousarray(Wo[sl, :]).astype(bf),
        })

    res = None
    for attempt in range(3):
        try:
            res = run_bass_kernel_spmd(nc, in_maps, core_ids=list(range(8)))
            break
        except Exception:
            # the axon-tunneled device occasionally reports
            # NRT_EXEC_UNIT_UNRECOVERABLE; the failure sticks to the PJRT
            # client, so tear down the backend to get a fresh worker
            if attempt == 2:
                raise
            import time
            import jax
            time.sleep(10)
            try:
                jax.clear_caches()
                jax.extend.backend.clear_backends()
            except Exception:
                pass
    ys = []
    for b in range(B):
        yt = (res.results[2 * b]["yT"].astype(np.float32)
              + res.results[2 * b + 1]["yT"].astype(np.float32))
        ys.append(yt.T + bo[None, :])
    return np.stack(ys, 0).astype(np.float32)


# revision 24
# speedup vs baseline: 1.2457x; 1.2457x over previous
"""nn_CrossAttention — Trainium2 Bass kernel (8 NeuronCores, SPMD), v2.

Sharding: core c handles batch b=c//2 and head-group g=c%2 (4 of 8 heads).
Host-side unshard sums the two head-group partials per batch, transposes,
and adds the output bias.

v2 engine plan (vs the f32r v1):
- Q/K/V/Y projections in bf16 (full PE rate, tiny error).
- Scores (Q.K^T) and attention.V in fp8e4 with DoubleRow perf mode
  (0.5 cycles/row).  Scores zero-pad the second contraction slot (dh=64);
  AV uses the two slots for adjacent key blocks (real 256-wide contraction).
- exp() split across engines: most tiles on Act (table exp), some via a
  Schraudolph bit-trick on DVE (f32 affine -> int16 == bf16 bits) with the
  bf16->fp8 quantize pass on GPSIMD/Pool.
- Softmax denominators from a ones column appended to V (row 64 of the AV
  PSUM accumulator); no max-subtraction (scores are O(1) here).
- y streamed directly PSUM -> DRAM (f32), x/weights DMA'd in bf16.
"""

from contextlib import ExitStack

import numpy as np
import ml_dtypes

import concourse.bass as bass
import concourse.mybir as mybir
import concourse.tile as tile
from concourse import bacc
from concourse.bass_utils import run_bass_kernel_spmd

F32 = mybir.dt.float32
BF16 = mybir.dt.bfloat16
FP8 = mybir.dt.float8e4
U8 = mybir.dt.uint8
F16 = mybir.dt.float16
DR = mybir.MatmulPerfMode.DoubleRow

B, H = 4, 8
N = 4096          # queries per batch
M = 1024          # keys
QD = 1024         # query dim
CD = 768          # context dim
DH = 64           # head dim
HL = 4            # heads per core
IL = HL * DH      # local inner = 256
SCALE = DH ** -0.5

QCH = 512         # query chunk (moving dim)
NCH = N // QCH
NKT = M // 128
NQT = QD // 128
NCT = CD // 128
NYT = QD // 128
VW = 80        # padded V width for dual-fp8 ldweights (multiple of 16)



def _build():
    nc = bacc.Bacc("TRN2", target_bir_lowering=False, debug=False)

    xT = nc.declare_dram_parameter("xT", [QD, N], BF16, isOutput=False)
    ctxT = nc.declare_dram_parameter("ctxT", [CD, M], BF16, isOutput=False)
    wq = nc.declare_dram_parameter("wq", [QD, IL], BF16, isOutput=False)
    wk = nc.declare_dram_parameter("wk", [CD, IL], BF16, isOutput=False)
    wv = nc.declare_dram_parameter("wv", [CD, IL], BF16, isOutput=False)
    wo = nc.declare_dram_parameter("wo", [IL, QD], BF16, isOutput=False)
    yT = nc.declare_dram_parameter("yT", [QD, N], BF16, isOutput=True)

    xT_r = xT.rearrange("(kt p) (c q) -> p kt c q", p=128, q=QCH)
    ctx_r = ctxT.rearrange("(ct p) m -> p ct m", p=128)
    wq_r = wq.rearrange("(kt p) i -> p kt i", p=128)
    wk_r = wk.rearrange("(ct p) i -> p ct i", p=128)
    wv_r = wv.rearrange("(ct p) i -> p ct i", p=128)
    wo_r = wo.rearrange("(it p) d -> p it d", p=128)
    yT_r = yT.rearrange("(yt p) (c q) -> p yt c q", p=128, q=QCH)

    with tile.TileContext(nc) as tc, ExitStack() as stack:
        sing = stack.enter_context(tc.tile_pool(name="sing", bufs=1))

        # ---- persistent tiles ----
        wq_sb = sing.tile([128, NQT, IL], BF16)
        nc.sync.dma_start(out=wq_sb, in_=wq_r)
        wo_sb = sing.tile([128, 2, QD], BF16)
        nc.sync.dma_start(out=wo_sb, in_=wo_r)
        # K^T in fp8: [dh-of-pair, mi(head pair), kt, slot, kcol]; the dual-fp8
        # Ldweights needs the two slots contiguous, so slot sits next to kcol.
        # Slot 1 is zeroed (scores contract only dh=64).
        kt8 = sing.tile([128, 2, NKT, 2, 128], FP8)
        nc.vector.memset(kt8[:, :, :, 1, :], 0.0)
        # V in fp16 with a ones column (softmax denom = AV psum row 64):
        # [key%128, kt, head, dh+1]
        vaug = sing.tile([128, NKT, HL, DH + 1], F16)
        ones32 = sing.tile([128, NKT, HL, 1], F32)
        nc.vector.memset(ones32, 1.0)
        nc.vector.tensor_copy(vaug[:, :, :, DH:DH + 1], ones32)
        # Q^T in fp8, manual double buffer: [dh-of-pair, buf, mi, slot, q]
        qt2 = sing.tile([128, 2, 2, 2, QCH], FP8)
        nc.vector.memset(qt2[:, :, :, 1, :], 0.0)

        # ---- stage B pools (opened up front; stage A shares ps2) ----
        stagea = stack.enter_context(tc.tile_pool(name="stagea", bufs=1))
        xpool = stack.enter_context(tc.tile_pool(name="xpool", bufs=2))
        ptp = stack.enter_context(tc.tile_pool(name="ptp", bufs=2))
        o2p = stack.enter_context(tc.tile_pool(name="o2p", bufs=3))
        ycp = stack.enter_context(tc.tile_pool(name="ycp", bufs=3))
        smallp = stack.enter_context(tc.tile_pool(name="smallp", bufs=2))
        ps2 = stack.enter_context(tc.tile_pool(name="ps2", bufs=3, space="PSUM"))
        pso = stack.enter_context(tc.tile_pool(name="pso", bufs=2, space="PSUM"))

        wk_sb = stagea.tile([128, NCT, IL], BF16)
        nc.sync.dma_start(out=wk_sb, in_=wk_r)
        wv_sb = stagea.tile([128, NCT, IL], BF16)
        nc.sync.dma_start(out=wv_sb, in_=wv_r)
        ctx_sb = stagea.tile([128, NCT, M], BF16)
        nc.sync.dma_start(out=ctx_sb, in_=ctx_r)

        # stage A part 1: K^T (needed by the first scores)
        for mi in range(2):
            pk = ps2.tile([128, 2, QCH], F32, tag="s")
            for nch2 in range(2):
                for ct in range(NCT):
                    nc.tensor.matmul(
                        pk[:, nch2, :], wk_sb[:, ct, mi * 128:(mi + 1) * 128],
                        ctx_sb[:, ct, nch2 * QCH:(nch2 + 1) * QCH],
                        start=(ct == 0), stop=(ct == NCT - 1))
            nc.vector.tensor_copy(
                kt8[:, mi, :, 0, :],
                pk.rearrange("p n (k c) -> p (n k) c", k=4))

        # ---- per-chunk stages, software-pipelined across chunks ----
        xcs, pts, o2ts = {}, {}, {}

        def q_proj(c):
            xc = xpool.tile([128, NQT, QCH], BF16)
            nc.sync.dma_start(out=xc, in_=xT_r[:, :, c, :])
            pq = ps2.tile([128, 2, QCH], F32, tag="s")
            for mi in range(2):
                for kt in range(NQT):
                    nc.tensor.matmul(
                        pq[:, mi, :], wq_sb[:, kt, mi * 128:(mi + 1) * 128],
                        xc[:, kt, :],
                        start=(kt == 0), stop=(kt == NQT - 1))
            nc.vector.tensor_copy(qt2[:, c % 2, :, 0, :], pq)
            o2a = o2p.tile([128, QCH], BF16, tag="o2a")
            o2b = o2p.tile([128, QCH], BF16, tag="o2b")
            o2ts[c] = [o2a, o2b]

        def scores_head(c, h):
            mi, hoff = h // 2, (h % 2) * 64
            pt = ptp.tile([128, NKT, QCH], F16)
            pts[(c, h)] = pt
            for j in range(4):  # kt pairs
                sc = ps2.tile([128, 2, QCH], F32, tag="s")
                for i in range(2):
                    kt = 2 * j + i
                    nc.tensor.matmul(
                        sc[:, i, :],
                        kt8[hoff:hoff + 64, mi, kt, :, :],
                        qt2[hoff:hoff + 64, c % 2, mi, :, :],
                        start=True, stop=True, perf_mode=DR)
                nc.scalar.activation(
                    pt[:, 2 * j:2 * j + 2, :], sc,
                    mybir.ActivationFunctionType.Exp, scale=SCALE)

        def av_head(c, h):
            hoff = (h % 2) * 64
            pt = pts.pop((c, h))
            po = pso.tile([DH + 1, QCH], F32)
            for kt in range(NKT):
                nc.tensor.matmul(
                    po, vaug[:, kt, h, :], pt[:, kt, :],
                    start=(kt == 0), stop=(kt == NKT - 1))
            ra = smallp.tile([1, QCH], F32, tag=f"ra{h % 2}")
            nc.vector.reciprocal(ra, po[DH:DH + 1, :])
            bca = smallp.tile([64, QCH], F32, tag=f"bc{h % 2}")
            nc.gpsimd.partition_broadcast(bca, ra)
            nc.vector.tensor_mul(o2ts[c][h // 2][hoff:hoff + 64, :],
                                 po[0:DH, :], bca)

        def y_phase(c):
            o2t = o2ts.pop(c)
            for yp in range(NYT // 2):
                py = ps2.tile([128, 2, QCH], F32, tag="s")
                for i in range(2):
                    yt = 2 * yp + i
                    nc.tensor.matmul(
                        py[:, i, :], wo_sb[:, 0, yt * 128:(yt + 1) * 128],
                        o2t[0], start=True, stop=False)
                    nc.tensor.matmul(
                        py[:, i, :], wo_sb[:, 1, yt * 128:(yt + 1) * 128],
                        o2t[1], start=False, stop=True)
                yc = ycp.tile([128, 2, QCH], BF16)
                nc.vector.tensor_copy(yc, py)
                nc.sync.dma_start(
                    out=yT_r[:, 2 * yp:2 * yp + 2, c, :], in_=yc)

        # prologue
        q_proj(0)
        scores_head(0, 0)
        # stage A part 2: V (overlaps the first chunk's scores/exp)
        for kp in range(NKT // 2):
            pv = ps2.tile([128, 2, QCH], F32, tag="s")
            for i in range(2):
                kt = 2 * kp + i
                for ct in range(NCT):
                    nc.tensor.matmul(
                        pv[:, i, 0:IL], ctx_sb[:, ct, kt * 128:(kt + 1) * 128],
                        wv_sb[:, ct, :],
                        start=(ct == 0), stop=(ct == NCT - 1))
                nc.vector.tensor_copy(
                    vaug[:, kt, :, 0:DH],
                    pv[:, i, 0:IL].rearrange("p (h d) -> p h d", h=HL))

        for c in range(NCH):
            for h in range(1, HL):
                scores_head(c, h)
                av_head(c, h - 1)
            if c + 1 < NCH:
                q_proj(c + 1)
            av_head(c, HL - 1)
            if c + 1 < NCH:
                scores_head(c + 1, 0)
            y_phase(c)

    nc.compile()
    return nc


_NC_CACHE = {}


def _get_nc():
    if "nc" not in _NC_CACHE:
        _NC_CACHE["nc"] = _build()
    return _NC_CACHE["nc"]


def kernel(x, context, Wq, Wk, Wv, Wo, bo):
    bf = ml_dtypes.bfloat16
    x = np.asarray(x, np.float32)
    context = np.asarray(context, np.float32)
    Wq = np.asarray(Wq, np.float32)
    Wk = np.asarray(Wk, np.float32)
    Wv = np.asarray(Wv, np.float32)
    Wo = np.asarray(Wo, np.float32)
    bo = np.asarray(bo, np.float32)

    nc = _get_nc()
    in_maps = []
    for c in range(8):
        b, g = c // 2, c % 2
        sl = slice(g * IL, (g + 1) * IL)
        in_maps.append({
            "xT": np.ascontiguousarray(x[b].T).astype(bf),
            "ctxT": np.ascontiguousarray(context[b].T).astype(bf),
            "wq": np.ascontiguousarray(Wq[:, sl]).astype(bf),
            "wk": np.ascontiguousarray(Wk[:, sl]).astype(bf),
            "wv": np.ascontiguousarray(Wv[:, sl]).astype(bf),
            "wo": np.ascontiguousarray(Wo[sl, :]).astype(bf),
        })

    res = None
    for attempt in range(3):
        try:
            res = run_bass_kernel_spmd(nc, in_maps, core_ids=list(range(8)))
            break
        except Exception:
            # the axon-tunneled device occasionally reports
            # NRT_EXEC_UNIT_UNRECOVERABLE; the failure sticks to the PJRT
            # client, so tear down the backend to get a fresh worker
            if attempt == 2:
                raise
            import time
            import jax
            time.sleep(10)
            try:
                jax.clear_caches()
                jax.extend.backend.clear_backends()
            except Exception:
                pass
    ys = []
    for b in range(B):
        yt = (res.results[2 * b]["yT"].astype(np.float32)
              + res.results[2 * b + 1]["yT"].astype(np.float32))
        ys.append(yt.T + bo[None, :])
    return np.stack(ys, 0).astype(np.float32)
